# revision 14
# baseline (speedup 1.0000x reference)
"""DIN attention layer kernel for Trainium2 (8 NeuronCores, data-parallel over batch).

Reference computation (per batch b):
    att = [q, k, q-k, q*k]            # [T, 4M]
    h1  = relu(att @ W1 + b1)         # [T, D]
    h2  = relu(h1 @ W2 + b2)          # [T, D]
    s   = h2 @ w_score + b_score      # [T, 1]
    attn = softmax(s.T + mask * -1e9) # [1, T]
    out = attn @ values               # [1, D]

Key optimizations:
  * Data-parallel: 8 batches per core (B=64 over 8 cores).
  * Masked-token compaction on the host: tokens with mask=1 get attention
    weight exactly 0 (exp(-1e9) underflows to 0 in fp32, identical to the
    reference softmax), so they are dropped before the device kernel.
    Batches are sorted by kept-token count and assigned round-robin, so
    batch-slot s runs with the exact max kept count of its rank group
    (degrades gracefully to dense T=1024 for any mask density).
  * Algebraic reassociation of the concat matmul:
        att @ W1 = q@(W1a+W1c) + k@(W1b-W1c) + (q*k)@W1d
    The q term is a per-batch row vector folded into the layer-1 bias;
    the q*k term is folded into the weights per batch:
        (q*k) @ W1d = k @ (diag(q) W1d)
    so layer 1 contracts over just M=256 (vs 1024 naively).
  * All tensors host-packed partition-major (one contiguous row per SBUF
    partition) and pre-cast to bf16 where applicable: minimal DMA
    descriptor counts, no device-side transposes or casts.
  * DMA issue spread over the three DGE queues (Sync, ACT, GpSimd).
  * Scores computed transposed ([token%128, tokchunk] layout) by using h2
    as the matmul stationary operand: softmax becomes small [128, tc]
    ops, attn lands in lhsT layout directly — no DRAM round-trip.
  * bf16 matmuls (fp32 PSUM accumulation); softmax in fp32.
  * Layer-1 PSUM drains alternate DVE/ACT so neither engine gates the PE.
  * attn@values for batch b is emitted inside batch b+1's mm2 loop so the
    PE never waits on the softmax chain.
"""

import numpy as np
import ml_dtypes

P = 128
B = 8          # batches per core
T = 1024       # tokens (full)
M = 256        # key feature dim
D = 1024       # hidden dim
MC = M // P    # key-feature chunks (2)
DC = D // P    # hidden chunks (8)
NEG = -1.0e9

BF16NP = ml_dtypes.bfloat16

_built = {}


def _chunks(tp):
    """Token free-dim chunks of width <= 512."""
    out = []
    off = 0
    while off < tp:
        w = min(512, tp - off)
        out.append((off, w))
        off += w
    return out


def _build(tps, dbg=False):
    import concourse.bass as bass  # noqa: F401
    import concourse.bacc as bacc
    import concourse.mybir as mybir
    import concourse.tile as tile
    from contextlib import ExitStack

    F32 = mybir.dt.float32
    BF16 = mybir.dt.bfloat16
    AF = mybir.ActivationFunctionType
    OP = mybir.AluOpType

    tcs = [-(-tp // P) for tp in tps]      # per-slot 128-token chunk counts
    tc_max = max(tcs)
    tp_pad = tc_max * P                    # padded token capacity (layouts)

    nc = bacc.Bacc("TRN2")
    # host-packed inputs: leading dim = SBUF partition, rows contiguous
    qtf_d = nc.dram_tensor("qt_f", [P, MC * B], F32, kind="ExternalInput").ap()
    qtb_d = nc.dram_tensor("qt_b", [P, MC * B], BF16, kind="ExternalInput").ap()
    b1_d = nc.dram_tensor("b1s", [P, DC], F32, kind="ExternalInput").ap()
    b2_d = nc.dram_tensor("b2s", [P, DC], F32, kind="ExternalInput").ap()
    ws_d = nc.dram_tensor("wss", [P, DC], BF16, kind="ExternalInput").ap()
    w1qc_d = nc.dram_tensor("w1qc", [P, MC * D], BF16, kind="ExternalInput").ap()
    w1bc_d = nc.dram_tensor("w1bc", [P, MC * D], BF16, kind="ExternalInput").ap()
    w1d_d = nc.dram_tensor("w1d", [P, MC * D], BF16, kind="ExternalInput").ap()
    w2_d = nc.dram_tensor("w2", [P, DC * D], BF16, kind="ExternalInput").ap()
    kt_d = nc.dram_tensor("keys_t", [B, P, MC * tp_pad], BF16,
                          kind="ExternalInput").ap()
    v_d = nc.dram_tensor("values", [B, P, tc_max * D], BF16,
                         kind="ExternalInput").ap()
    m_d = nc.dram_tensor("mask_r", [B, tc_max * P], F32, kind="ExternalInput").ap()
    out_d = nc.dram_tensor("out", [B, D], F32, kind="ExternalOutput").ap()
    dbg_d = None
    if dbg:
        dbg_d = nc.dram_tensor("dbg", [B, 1282], F32, kind="ExternalOutput").ap()

    with tile.TileContext(nc) as tc, ExitStack() as ctx:
        cons = ctx.enter_context(tc.tile_pool(name="cons", bufs=1))
        xpool = ctx.enter_context(tc.tile_pool(name="xpool", bufs=2))
        wepool = ctx.enter_context(tc.tile_pool(name="wep", bufs=2))
        vpool = ctx.enter_context(tc.tile_pool(name="vp", bufs=2))
        small = ctx.enter_context(tc.tile_pool(name="small", bufs=2))
        psum_mm = ctx.enter_context(tc.tile_pool(name="psmm", bufs=4, space="PSUM"))
        psum_sc = ctx.enter_context(tc.tile_pool(name="pssc", bufs=2, space="PSUM"))
        psum_vec = ctx.enter_context(tc.tile_pool(name="psvec", bufs=2, space="PSUM"))

        # ---- one-time setup -------------------------------------------------
        # critical path first: qt + w1bc/w1d (w1eff deps) on the Sync HWDGE
        # queue; w1qc/w2 on the ACT HWDGE queue; the rest on GpSimd SWDGE.
        qt_f = cons.tile([P, MC, B], F32)
        qt_b = cons.tile([P, MC, B], BF16)
        nc.sync.dma_start(qt_f, qtf_d.rearrange("p (c b) -> p c b", c=MC))
        nc.sync.dma_start(qt_b, qtb_d.rearrange("p (c b) -> p c b", c=MC))
        b1_sb = cons.tile([P, DC], F32)
        nc.sync.dma_start(b1_sb, b1_d)
        w1bc = cons.tile([P, MC, D], BF16)
        w1d_sb = cons.tile([P, MC, D], BF16)
        nc.sync.dma_start(w1bc, w1bc_d.rearrange("p (c d) -> p c d", c=MC))
        nc.sync.dma_start(w1d_sb, w1d_d.rearrange("p (c d) -> p c d", c=MC))
        w1qc = cons.tile([P, MC, D], BF16)
        nc.scalar.dma_start(w1qc, w1qc_d.rearrange("p (c d) -> p c d", c=MC))
        w2_sb = cons.tile([P, DC, D], BF16)
        nc.scalar.dma_start(w2_sb, w2_d.rearrange("p (c d) -> p c d", c=DC))
        b2_sb = cons.tile([P, DC], F32)
        ws_sb = cons.tile([P, DC], BF16)
        nc.gpsimd.dma_start(b2_sb, b2_d)
        nc.gpsimd.dma_start(ws_sb, ws_d)

        ones_bf = cons.tile([P, 1], BF16)
        nc.vector.memset(ones_bf, 1.0)

        # fixed activation buffers (reused across batches); exp-row tails
        # beyond each slot's exact token count are read by the attn
        # transposes — zero them once so they are always finite (their
        # attn@values contribution is 0 because pad values rows are 0)
        h1buf = cons.tile([P, DC, tp_pad], BF16)
        h2buf = [cons.tile([P, tp_pad], BF16, name=f"h2_{i}") for i in range(2)]
        expbuf = [cons.tile([1, tp_pad], BF16, name=f"exp_{i}") for i in range(2)]
        for i in range(2):
            nc.vector.memset(h2buf[i], 0.0)
            nc.vector.memset(expbuf[i], 0.0)

        # RT[p, b, j] = (q_b @ (W1a+W1c))[j*128+p] + b1[j*128+p], on the PE
        rt = cons.tile([P, B, DC], F32)
        for j in range(DC):
            rt_ps = psum_vec.tile([P, B], F32, tag="vec", name=f"rt_ps{j}")
            for c in range(MC):
                nc.tensor.matmul(
                    rt_ps, w1qc[:, c, j * P:(j + 1) * P], qt_b[:, c, :],
                    start=(c == 0), stop=(c == MC - 1),
                )
            nc.vector.tensor_scalar(
                rt[:, :, j], rt_ps, b1_sb[:, j:j + 1], None, op0=OP.add,
            )

        # ---- per-batch pipeline --------------------------------------------
        carry = {}

        def emit_attn_values(b):
            st = carry.pop(b)
            tcn = st["tcn"]
            # transpose the exp row into lhsT layout on the PE: each chunk is
            # an atomic start+stop matmul against [[1]] into its own bank
            attn_t = small.tile([P, tcn], BF16, tag="attn")
            for c in range(tcn):
                tp_ps = psum_vec.tile([P, 1], F32, tag="vec", name=f"t_ps{c}")
                nc.tensor.matmul(
                    tp_ps, st["exp"][0:1, c * P:(c + 1) * P], ones_bf[0:1, 0:1],
                    start=True, stop=True,
                )
                nc.vector.tensor_copy(attn_t[:, c:c + 1], tp_ps)
            out_ps = [psum_vec.tile([1, 512], F32, tag="vec", name=f"o_ps{h}")
                      for h in range(2)]
            for h in range(2):
                for c in range(tcn):
                    nc.tensor.matmul(
                        out_ps[h],
                        attn_t[:, c:c + 1],
                        st["vals"][:, c, h * 512:(h + 1) * 512],
                        start=(c == 0), stop=(c == tcn - 1),
                    )
            out_sb = small.tile([1, D], F32, tag="osb")
            for h in range(2):
                nc.vector.tensor_scalar_mul(
                    out_sb[:, h * 512:(h + 1) * 512], out_ps[h], st["rec"])
            nc.gpsimd.dma_start(out_d[b:b + 1, :], out_sb)

        for b in range(B):
            tp = tps[b]
            tcn = tcs[b]
            cks = _chunks(tp)

            # per-batch effective layer-1 weights: W1eff = W1bc + q_b * W1d
            w1eff = wepool.tile([P, MC, D], BF16, tag="weff")
            for c in range(MC):
                nc.vector.scalar_tensor_tensor(
                    w1eff[:, c, :], in0=w1d_sb[:, c, :], scalar=qt_f[:, c, b:b + 1],
                    in1=w1bc[:, c, :], op0=OP.mult, op1=OP.add,
                )

            x_t = xpool.tile([P, MC, tp], BF16, tag="X")
            nc.sync.dma_start(
                x_t, kt_d[b].rearrange("p (c t) -> p c t", t=tp_pad)[:, :, 0:tp])
            mask_t = small.tile([1, tp], F32, tag="mask")
            nc.gpsimd.dma_start(mask_t, m_d[b:b + 1, 0:tp])
            vals = vpool.tile([P, tcn, D], BF16, tag="vals")
            nc.scalar.dma_start(
                vals, v_d[b].rearrange("p (to d) -> p to d", d=D)[:, 0:tcn, :])

            # mm1: H1[d, t] = relu(W1eff.T @ X + rt_b)
            for j in range(DC):
                for off, w in cks:
                    ps = psum_mm.tile([P, w], F32, tag="mm", name=f"m1_{j}_{off}")
                    for c in range(MC):
                        nc.tensor.matmul(
                            ps, w1eff[:, c, j * P:(j + 1) * P],
                            x_t[:, c, off:off + w],
                            start=(c == 0), stop=(c == MC - 1),
                        )
                    if j % 2 == 0:
                        nc.vector.tensor_scalar(
                            h1buf[:, j, off:off + w], ps, rt[:, b, j:j + 1], 0.0,
                            op0=OP.add, op1=OP.max,
                        )
                    else:
                        nc.scalar.activation(
                            h1buf[:, j, off:off + w], ps, AF.Relu,
                            bias=rt[:, b, j:j + 1], scale=1.0,
                        )

            # mm2 + score: one accumulation group per [1, w] psum tile
            # (deferred by one j so the PE doesn't wait on the relu)
            score_ps = [psum_sc.tile([1, w], F32, tag="sc", name=f"s_ps{off}")
                        for off, w in cks]

            def emit_score(jj):
                h2p = h2buf[jj % 2]
                for ci, (off, w) in enumerate(cks):
                    nc.tensor.matmul(
                        score_ps[ci], ws_sb[:, jj:jj + 1], h2p[:, off:off + w],
                        start=(jj == 0), stop=(jj == DC - 1),
                        skip_group_check=True,
                    )

            for j in range(DC):
                h2 = h2buf[j % 2]
                for off, w in cks:
                    ps = psum_mm.tile([P, w], F32, tag="mm", name=f"m2_{j}_{off}")
                    for c in range(DC):
                        nc.tensor.matmul(
                            ps, w2_sb[:, c, j * P:(j + 1) * P],
                            h1buf[:, c, off:off + w],
                            start=(c == 0), stop=(c == DC - 1),
                        )
                    nc.scalar.activation(
                        h2[:, off:off + w], ps, AF.Relu,
                        bias=b2_sb[:, j:j + 1], scale=1.0,
                    )
                if j > 0:
                    emit_score(j - 1)
                # deferred attn@values for the previous batch: emitted behind
                # mm1 + one mm2 j-round of PE work so its softmax chain is
                # fully hidden
                if j == 1 and b > 0:
                    emit_attn_values(b - 1)
            emit_score(DC - 1)

            # softmax (no max subtraction: scores are O(1), masked/pad lanes
            # underflow to exactly 0). score = mask * -1e9 + raw_score
            score_sb = small.tile([1, tp], F32, tag="ssb")
            for ci, (off, w) in enumerate(cks):
                nc.vector.scalar_tensor_tensor(
                    score_sb[:, off:off + w], in0=mask_t[:, off:off + w],
                    scalar=NEG, in1=score_ps[ci], op0=OP.mult, op1=OP.add,
                )
            sum_sb = small.tile([1, 1], F32, tag="sum")
            exp_row = expbuf[b % 2]
            nc.scalar.activation(
                exp_row[:, 0:tp], score_sb, AF.Exp, accum_out=sum_sb)
            rec = small.tile([1, 1], F32, tag="rec")
            nc.vector.reciprocal(rec, sum_sb)

            if dbg:
                nc.gpsimd.dma_start(dbg_d[b:b + 1, 0:tp], score_sb)
                nc.gpsimd.dma_start(dbg_d[b:b + 1, 1280:1281], sum_sb)
                nc.gpsimd.dma_start(dbg_d[b:b + 1, 1281:1282], rec)

            carry[b] = {"exp": exp_row, "vals": vals, "rec": rec, "tcn": tcn}

        emit_attn_values(B - 1)

    nc.compile()
    return nc


def _get_built(tps):
    key = tuple(tps)
    if key not in _built:
        _built[key] = _build(key)
    return _built[key]


N_CORES = 8


def _pack_rows(a, c):
    """[c*P, N] -> [P, c*N] with row p = concat_c a[c*P + p, :]."""
    n = a.shape[1]
    return np.ascontiguousarray(
        a.reshape(c, P, n).transpose(1, 0, 2).reshape(P, c * n))


def _prep(query, keys, values, mask, W1, b1, W2, b2, w_score):
    """Host-side: compaction + sorted slot assignment + layout/dtype prep."""
    query = np.asarray(query, dtype=np.float32).reshape(64, M)
    keys = np.asarray(keys, dtype=np.float32)
    values = np.asarray(values, dtype=np.float32)
    mask = np.asarray(mask, dtype=np.float32).reshape(64, T)

    kept = [np.flatnonzero(mask[i] < 0.5) for i in range(64)]
    # sort batches by kept count desc; slot s of core c <- rank (s*8 + c)
    order = np.argsort([-len(k) for k in kept], kind="stable")
    tps, tcs = [], []
    for s in range(B):
        grp = order[s * N_CORES:(s + 1) * N_CORES]
        tp = min(T, max(1, max(len(kept[g]) for g in grp)))
        tcn = -(-tp // P)
        tps.append(tp)
        tcs.append(tcn)
    tc_max = max(tcs)
    tp_pad = tc_max * P

    keys_t = np.zeros((64, P, MC * tp_pad), dtype=BF16NP)
    vals_c = np.zeros((64, P, tc_max * D), dtype=BF16NP)
    mask_c = np.ones((64, tc_max * P), dtype=np.float32)
    qt = np.zeros((64, M), dtype=np.float32)
    slot_of = {}
    for s in range(B):
        for c in range(N_CORES):
            g = order[s * N_CORES + c]
            slot_of[(c, s)] = g
            idx = kept[g]
            n = len(idx)
            i = c * B + s  # row in the packed per-core arrays
            kT = np.zeros((M, tp_pad), dtype=np.float32)
            kT[:, :n] = keys[g, idx, :].T
            keys_t[i] = _pack_rows(kT, MC).astype(BF16NP)
            v = np.zeros((tc_max * P, D), dtype=np.float32)
            v[:n] = values[g, idx, :]
            vals_c[i] = _pack_rows(v, tc_max).astype(BF16NP)
            mask_c[i, :n] = 0.0
            qt[i] = query[g]

    W1 = np.asarray(W1, dtype=np.float32)
    w1qc = _pack_rows(W1[0:M] + W1[2 * M:3 * M], MC).astype(BF16NP)
    w1bc = _pack_rows(W1[M:2 * M] - W1[2 * M:3 * M], MC).astype(BF16NP)
    w1d = _pack_rows(W1[3 * M:4 * M], MC).astype(BF16NP)
    w2p = _pack_rows(np.asarray(W2, dtype=np.float32), DC).astype(BF16NP)

    def stripe(v):
        return np.ascontiguousarray(
            np.asarray(v, dtype=np.float32).reshape(-1)[: D].reshape(DC, P).T)

    b1s = stripe(b1)
    b2s = stripe(b2)
    wss = stripe(w_score).astype(BF16NP)

    shared = {
        "w1qc": w1qc, "w1bc": w1bc, "w1d": w1d, "w2": w2p,
        "b1s": b1s, "b2s": b2s, "wss": wss,
    }
    in_maps = []
    for c in range(N_CORES):
        rows = [c * B + s for s in range(B)]
        q_core = qt[rows]  # [B, M]
        qt_f = np.ascontiguousarray(
            q_core.reshape(B, MC, P).transpose(2, 1, 0).reshape(P, MC * B))
        in_maps.append({
            "qt_f": qt_f,
            "qt_b": qt_f.astype(BF16NP),
            "keys_t": np.ascontiguousarray(keys_t[rows]),
            "values": np.ascontiguousarray(vals_c[rows]),
            "mask_r": np.ascontiguousarray(mask_c[rows]),
            **shared,
        })
    return tps, slot_of, in_maps


def make_in_maps(query, keys, values, mask, W1, b1, W2, b2, w_score, b_score=None):
    # b_score is ignored: softmax is shift-invariant.
    return _prep(query, keys, values, mask, W1, b1, W2, b2, w_score)


def gather_out(results, slot_of):
    out = np.empty((64, 1, D), dtype=np.float32)
    for c in range(N_CORES):
        for s in range(B):
            out[slot_of[(c, s)], 0, :] = results[c]["out"][s]
    return out


def kernel(query, keys, values, mask, W1, b1, W2, b2, w_score, b_score):
    """Full-input entry point: shards over 8 NeuronCores, returns [64, 1, D]."""
    from concourse.bass_utils import run_bass_kernel_spmd

    tps, slot_of, in_maps = _prep(
        query, keys, values, mask, W1, b1, W2, b2, w_score)
    nc = _get_built(tps)
    res = run_bass_kernel_spmd(nc, in_maps, core_ids=list(range(N_CORES)))
    return gather_out(res.results, slot_of)


# revision 19
# speedup vs baseline: 1.0805x; 1.0805x over previous
"""DIN attention layer kernel for Trainium2 (8 NeuronCores, data-parallel over batch).

Reference computation (per batch b):
    att = [q, k, q-k, q*k]            # [T, 4M]
    h1  = relu(att @ W1 + b1)         # [T, D]
    h2  = relu(h1 @ W2 + b2)          # [T, D]
    s   = h2 @ w_score + b_score      # [T, 1]
    attn = softmax(s.T + mask * -1e9) # [1, T]
    out = attn @ values               # [1, D]

Key optimizations:
  * Data-parallel: 8 batches per core (B=64 over 8 cores).
  * Masked-token compaction on the host: tokens with mask=1 get attention
    weight exactly 0 (exp(-1e9) underflows to 0 in fp32, identical to the
    reference softmax), so they are dropped before the device kernel.
    Batches are sorted by kept-token count and assigned round-robin, so
    batch-slot s runs with the exact max kept count of its rank group
    (degrades gracefully to dense T=1024 for any mask density).
  * Algebraic reassociation of the concat matmul:
        att @ W1 = q@(W1a+W1c) + k@(W1b-W1c) + (q*k)@W1d
    The q term is a per-batch row vector folded into the layer-1 bias;
    the q*k term is folded into the weights per batch:
        (q*k) @ W1d = k @ (diag(q) W1d)
    so layer 1 contracts over just M=256 (vs 1024 naively).
  * All tensors host-packed partition-major and pre-cast to bf16 where
    applicable; weights j-interleaved and DMA'd in slices so matmul round
    j only waits for its own slice; DMA issue spread over Sync/ACT HWDGE
    queues with the late-needed bulk (values) on the GpSimd SWDGE path.
  * score = ws^T relu(...) folded on the DVE: acc += ws_j * h2_j chunk by
    chunk, then a single ones-vector reduce matmul per 512-token chunk
    (PE cost 1x tokens instead of 8x).
  * bf16 matmuls (fp32 PSUM accumulation); softmax in fp32.
  * Layer-1 PSUM drains alternate DVE/ACT so neither engine gates the PE.
  * The softmax tail of batch b (reduce, exp, transpose to lhsT layout,
    attn@values) is emitted inside batch b+1's compute so the PE never
    waits on it.
"""

import numpy as np
import ml_dtypes

P = 128
B = 8          # batches per core
T = 1024       # tokens (full)
M = 256        # key feature dim
D = 1024       # hidden dim
MC = M // P    # key-feature chunks (2)
DC = D // P    # hidden chunks (8)
NEG = -1.0e9

BF16NP = ml_dtypes.bfloat16

_built = {}


def _chunks(tp):
    """Token free-dim chunks of width <= 512."""
    out = []
    off = 0
    while off < tp:
        w = min(512, tp - off)
        out.append((off, w))
        off += w
    return out


def _build(tps, dbg=False):
    import concourse.bass as bass  # noqa: F401
    import concourse.bacc as bacc
    import concourse.mybir as mybir
    import concourse.tile as tile
    from contextlib import ExitStack

    F32 = mybir.dt.float32
    BF16 = mybir.dt.bfloat16
    AF = mybir.ActivationFunctionType
    OP = mybir.AluOpType

    tcs = [-(-tp // P) for tp in tps]      # per-slot 128-token chunk counts
    tc_max = max(tcs)
    tp_pad = tc_max * P                    # padded token capacity (layouts)

    nc = bacc.Bacc("TRN2")
    # host-packed inputs: leading dim = SBUF partition, rows contiguous;
    # weights j-interleaved: w[p, j, c, k] = W[c*128+p, j*128+k]
    qtf_d = nc.dram_tensor("qt_f", [P, MC * B], F32, kind="ExternalInput").ap()
    qtb_d = nc.dram_tensor("qt_b", [P, MC * B], BF16, kind="ExternalInput").ap()
    b1_d = nc.dram_tensor("b1s", [P, DC], F32, kind="ExternalInput").ap()
    b2_d = nc.dram_tensor("b2s", [P, DC], F32, kind="ExternalInput").ap()
    ws_d = nc.dram_tensor("wss", [P, DC], F32, kind="ExternalInput").ap()
    w1qc_d = nc.dram_tensor("w1qc", [P, DC * MC * P], BF16, kind="ExternalInput").ap()
    w1bc_d = nc.dram_tensor("w1bc", [P, DC * MC * P], BF16, kind="ExternalInput").ap()
    w1d_d = nc.dram_tensor("w1d", [P, DC * MC * P], BF16, kind="ExternalInput").ap()
    w2_d = nc.dram_tensor("w2", [P, DC * DC * P], BF16, kind="ExternalInput").ap()
    kt_d = nc.dram_tensor("keys_t", [B, P, MC * tp_pad], BF16,
                          kind="ExternalInput").ap()
    v_d = nc.dram_tensor("values", [B, P, tc_max * D], BF16,
                         kind="ExternalInput").ap()
    m_d = nc.dram_tensor("mask_r", [B, tc_max * P], F32, kind="ExternalInput").ap()
    out_d = nc.dram_tensor("out", [B, D], F32, kind="ExternalOutput").ap()
    dbg_d = None
    if dbg:
        dbg_d = nc.dram_tensor("dbg", [B, 1282], F32, kind="ExternalOutput").ap()

    w1qc_r = w1qc_d.rearrange("p (j c k) -> p j c k", j=DC, c=MC)
    w1bc_r = w1bc_d.rearrange("p (j c k) -> p j c k", j=DC, c=MC)
    w1d_r = w1d_d.rearrange("p (j c k) -> p j c k", j=DC, c=MC)
    w2_r = w2_d.rearrange("p (j c k) -> p j c k", j=DC, c=DC)

    with tile.TileContext(nc) as tc, ExitStack() as ctx:
        cons = ctx.enter_context(tc.tile_pool(name="cons", bufs=1))
        xpool = ctx.enter_context(tc.tile_pool(name="xpool", bufs=2))
        wepool = ctx.enter_context(tc.tile_pool(name="wep", bufs=2))
        vpool = ctx.enter_context(tc.tile_pool(name="vp", bufs=3))
        small = ctx.enter_context(tc.tile_pool(name="small", bufs=2))
        psum_mm = ctx.enter_context(tc.tile_pool(name="psmm", bufs=4, space="PSUM"))
        psum_sc = ctx.enter_context(tc.tile_pool(name="pssc", bufs=2, space="PSUM"))
        psum_vec = ctx.enter_context(tc.tile_pool(name="psvec", bufs=2, space="PSUM"))

        # ---- one-time setup -------------------------------------------------
        # Sync HWDGE: the layer-1 critical path, with batch-0 keys slotted
        # between the weight halves so mm1(b0) can start early.
        qt_f = cons.tile([P, MC, B], F32)
        qt_b = cons.tile([P, MC, B], BF16)
        nc.sync.dma_start(qt_f, qtf_d.rearrange("p (c b) -> p c b", c=MC))
        nc.sync.dma_start(qt_b, qtb_d.rearrange("p (c b) -> p c b", c=MC))
        b1_sb = cons.tile([P, DC], F32)
        nc.sync.dma_start(b1_sb, b1_d)
        w1bc = cons.tile([P, DC, MC, P], BF16)
        w1d_sb = cons.tile([P, DC, MC, P], BF16)
        HJ = DC // 2
        nc.sync.dma_start(w1d_sb[:, 0:HJ], w1d_r[:, 0:HJ])
        nc.sync.dma_start(w1bc[:, 0:HJ], w1bc_r[:, 0:HJ])

        x_pre = xpool.tile([P, MC, tps[0]], BF16, tag="X")
        nc.sync.dma_start(
            x_pre, kt_d[0].rearrange("p (c t) -> p c t", t=tp_pad)[:, :, 0:tps[0]])

        nc.sync.dma_start(w1d_sb[:, HJ:DC], w1d_r[:, HJ:DC])
        nc.sync.dma_start(w1bc[:, HJ:DC], w1bc_r[:, HJ:DC])

        # ACT HWDGE: rt weights + layer-2 weights (per-j slices) + vectors
        w1qc = cons.tile([P, DC, MC, P], BF16)
        nc.scalar.dma_start(w1qc, w1qc_r)
        b2_sb = cons.tile([P, DC], F32)
        ws_sb = cons.tile([P, DC], F32)
        nc.scalar.dma_start(b2_sb, b2_d)
        nc.scalar.dma_start(ws_sb, ws_d)
        w2_sb = cons.tile([P, DC, DC, P], BF16)
        for j in range(DC):
            nc.scalar.dma_start(w2_sb[:, j], w2_r[:, j])

        ones_bf = cons.tile([P, 1], BF16)
        nc.vector.memset(ones_bf, 1.0)

        # fixed activation buffers (reused across batches); exp-row tails
        # beyond each slot's exact token count are read by the attn
        # transposes — zero them once so they are always finite (their
        # attn@values contribution is 0 because pad values rows are 0)
        h1buf = cons.tile([P, DC, tp_pad], BF16)
        h2buf = [cons.tile([P, tp_pad], BF16, name=f"h2_{i}") for i in range(2)]
        accbuf = [cons.tile([P, tp_pad], BF16, name=f"acc_{i}") for i in range(2)]
        expbuf = [cons.tile([1, tp_pad], BF16, name=f"exp_{i}") for i in range(2)]
        for i in range(2):
            nc.vector.memset(expbuf[i], 0.0)

        # RT[p, b, j] = (q_b @ (W1a+W1c))[j*128+p] + b1[j*128+p], on the PE
        rt = cons.tile([P, B, DC], F32)
        for j in range(DC):
            rt_ps = psum_vec.tile([P, B], F32, tag="vec", name=f"rt_ps{j}")
            for c in range(MC):
                nc.tensor.matmul(
                    rt_ps, w1qc[:, j, c, :], qt_b[:, c, :],
                    start=(c == 0), stop=(c == MC - 1),
                )
            nc.vector.tensor_scalar(
                rt[:, :, j], rt_ps, b1_sb[:, j:j + 1], None, op0=OP.add,
            )

        # ---- per-batch pipeline --------------------------------------------
        soft = {}   # batch -> state after mm2 (awaiting reduce+softmax)
        carry = {}  # batch -> state after softmax (awaiting attn@values)

        def emit_softmax(b):
            st = soft.pop(b)
            tp = tps[b]
            cks = _chunks(tp)
            acc = st["acc"]
            score_ps = [psum_sc.tile([1, w], F32, tag="sc", name=f"s_ps{off}")
                        for off, w in cks]
            for ci, (off, w) in enumerate(cks):
                nc.tensor.matmul(
                    score_ps[ci], ones_bf, acc[:, off:off + w],
                    start=True, stop=True,
                )
            score_sb = small.tile([1, tp], F32, tag="ssb")
            for ci, (off, w) in enumerate(cks):
                nc.vector.scalar_tensor_tensor(
                    score_sb[:, off:off + w], in0=st["mask"][:, off:off + w],
                    scalar=NEG, in1=score_ps[ci], op0=OP.mult, op1=OP.add,
                )
            sum_sb = small.tile([1, 1], F32, tag="sum")
            exp_row = expbuf[b % 2]
            nc.scalar.activation(
                exp_row[:, 0:tp], score_sb, AF.Exp, accum_out=sum_sb)
            rec = small.tile([1, 1], F32, tag="rec")
            nc.vector.reciprocal(rec, sum_sb)
            if dbg:
                nc.gpsimd.dma_start(dbg_d[b:b + 1, 0:tp], score_sb)
                nc.gpsimd.dma_start(dbg_d[b:b + 1, 1280:1281], sum_sb)
                nc.gpsimd.dma_start(dbg_d[b:b + 1, 1281:1282], rec)
            carry[b] = {"exp": exp_row, "vals": st["vals"], "rec": rec,
                        "tcn": st["tcn"]}

        def emit_attn_values(b):
            st = carry.pop(b)
            tcn = st["tcn"]
            # transpose the exp row into lhsT layout on the PE: each chunk is
            # an atomic start+stop matmul against [[1]] into its own bank
            attn_t = small.tile([P, tcn], BF16, tag="attn")
            for c in range(tcn):
                tp_ps = psum_vec.tile([P, 1], F32, tag="vec", name=f"t_ps{c}")
                nc.tensor.matmul(
                    tp_ps, st["exp"][0:1, c * P:(c + 1) * P], ones_bf[0:1, 0:1],
                    start=True, stop=True,
                )
                nc.vector.tensor_copy(attn_t[:, c:c + 1], tp_ps)
            out_ps = [psum_vec.tile([1, 512], F32, tag="vec", name=f"o_ps{h}")
                      for h in range(2)]
            for h in range(2):
                for c in range(tcn):
                    nc.tensor.matmul(
                        out_ps[h],
                        attn_t[:, c:c + 1],
                        st["vals"][:, c, h * 512:(h + 1) * 512],
                        start=(c == 0), stop=(c == tcn - 1),
                    )
            out_sb = small.tile([1, D], F32, tag="osb")
            for h in range(2):
                nc.vector.tensor_scalar_mul(
                    out_sb[:, h * 512:(h + 1) * 512], out_ps[h], st["rec"])
            nc.scalar.dma_start(out_d[b:b + 1, :], out_sb)

        for b in range(B):
            tp = tps[b]
            tcn = tcs[b]
            cks = _chunks(tp)

            # per-batch effective layer-1 weights: W1eff = W1bc + q_b * W1d
            # (split by j-half so batch 0 can start on the first DMA half)
            w1eff = wepool.tile([P, DC, MC, P], BF16, tag="weff")
            for hj in range(2):
                js = slice(hj * HJ, (hj + 1) * HJ)
                for c in range(MC):
                    nc.vector.scalar_tensor_tensor(
                        w1eff[:, js, c, :], in0=w1d_sb[:, js, c, :],
                        scalar=qt_f[:, c, b:b + 1],
                        in1=w1bc[:, js, c, :], op0=OP.mult, op1=OP.add,
                    )

            if b == 0:
                x_t = x_pre
            else:
                x_t = xpool.tile([P, MC, tp], BF16, tag="X")
                nc.sync.dma_start(
                    x_t,
                    kt_d[b].rearrange("p (c t) -> p c t", t=tp_pad)[:, :, 0:tp])
            mask_t = small.tile([1, tp], F32, tag="mask")
            nc.gpsimd.dma_start(mask_t, m_d[b:b + 1, 0:tp])
            vals = vpool.tile([P, tcn, D], BF16, tag="vals")
            nc.gpsimd.dma_start(
                vals, v_d[b].rearrange("p (to d) -> p to d", d=D)[:, 0:tcn, :])

            # mm1: H1[d, t] = relu(W1eff.T @ X + rt_b)
            for j in range(DC):
                for off, w in cks:
                    ps = psum_mm.tile([P, w], F32, tag="mm", name=f"m1_{j}_{off}")
                    for c in range(MC):
                        nc.tensor.matmul(
                            ps, w1eff[:, j, c, :],
                            x_t[:, c, off:off + w],
                            start=(c == 0), stop=(c == MC - 1),
                        )
                    if j % 2 == 0:
                        nc.vector.tensor_scalar(
                            h1buf[:, j, off:off + w], ps, rt[:, b, j:j + 1], 0.0,
                            op0=OP.add, op1=OP.max,
                        )
                    else:
                        nc.scalar.activation(
                            h1buf[:, j, off:off + w], ps, AF.Relu,
                            bias=rt[:, b, j:j + 1], scale=1.0,
                        )

            # deferred softmax for the previous batch: its score reduce sits
            # behind mm1(b) on the PE so the acc chain has completed
            if b > 0:
                emit_softmax(b - 1)

            # mm2 + DVE score fold: acc = sum_j ws_j * relu(mm2_j + b2_j)
            for j in range(DC):
                h2 = h2buf[j % 2]
                for off, w in cks:
                    ps = psum_mm.tile([P, w], F32, tag="mm", name=f"m2_{j}_{off}")
                    for c in range(DC):
                        nc.tensor.matmul(
                            ps, w2_sb[:, j, c, :],
                            h1buf[:, c, off:off + w],
                            start=(c == 0), stop=(c == DC - 1),
                        )
                    nc.scalar.activation(
                        h2[:, off:off + w], ps, AF.Relu,
                        bias=b2_sb[:, j:j + 1], scale=1.0,
                    )
                if j == 0:
                    nc.vector.tensor_scalar_mul(
                        accbuf[0][:, 0:tp], h2[:, 0:tp], ws_sb[:, 0:1])
                else:
                    nc.vector.scalar_tensor_tensor(
                        accbuf[j % 2][:, 0:tp], in0=h2[:, 0:tp],
                        scalar=ws_sb[:, j:j + 1],
                        in1=accbuf[(j - 1) % 2][:, 0:tp],
                        op0=OP.mult, op1=OP.add,
                    )
                # deferred attn@values for the batch before: emitted behind
                # mm1 + one mm2 j-round of PE work so its softmax chain is
                # fully hidden
                if j == 1 and b > 1:
                    emit_attn_values(b - 2)

            soft[b] = {"acc": accbuf[(DC - 1) % 2], "mask": mask_t,
                       "vals": vals, "tcn": tcn}

        emit_softmax(B - 1)
        emit_attn_values(B - 2)
        emit_attn_values(B - 1)

    nc.compile()
    return nc


def _get_built(tps):
    key = tuple(tps)
    if key not in _built:
        _built[key] = _build(key)
    return _built[key]


N_CORES = 8


def _pack_rows(a, c):
    """[c*P, N] -> [P, c*N] with row p = concat_c a[c*P + p, :]."""
    n = a.shape[1]
    return np.ascontiguousarray(
        a.reshape(c, P, n).transpose(1, 0, 2).reshape(P, c * n))


def _pack_jc(a, nc_):
    """[nc_*P, D] -> [P, DC*nc_*P] with w[p, j, c, k] = a[c*P+p, j*P+k]."""
    return np.ascontiguousarray(
        a.reshape(nc_, P, DC, P).transpose(1, 2, 0, 3).reshape(P, DC * nc_ * P))


def _prep(query, keys, values, mask, W1, b1, W2, b2, w_score):
    """Host-side: compaction + sorted slot assignment + layout/dtype prep."""
    query = np.asarray(query, dtype=np.float32).reshape(64, M)
    keys = np.asarray(keys, dtype=np.float32)
    values = np.asarray(values, dtype=np.float32)
    mask = np.asarray(mask, dtype=np.float32).reshape(64, T)

    kept = [np.flatnonzero(mask[i] < 0.5) for i in range(64)]
    # sort batches by kept count desc; slot s of core c <- rank (s*8 + c)
    order = np.argsort([-len(k) for k in kept], kind="stable")
    tps, tcs = [], []
    for s in range(B):
        grp = order[s * N_CORES:(s + 1) * N_CORES]
        tp = min(T, max(1, max(len(kept[g]) for g in grp)))
        tcn = -(-tp // P)
        tps.append(tp)
        tcs.append(tcn)
    tc_max = max(tcs)
    tp_pad = tc_max * P

    keys_t = np.zeros((64, P, MC * tp_pad), dtype=BF16NP)
    vals_c = np.zeros((64, P, tc_max * D), dtype=BF16NP)
    mask_c = np.ones((64, tc_max * P), dtype=np.float32)
    qt = np.zeros((64, M), dtype=np.float32)
    slot_of = {}
    for s in range(B):
        for c in range(N_CORES):
            g = order[s * N_CORES + c]
            slot_of[(c, s)] = g
            idx = kept[g]
            n = len(idx)
            i = c * B + s  # row in the packed per-core arrays
            kT = np.zeros((M, tp_pad), dtype=np.float32)
            kT[:, :n] = keys[g, idx, :].T
            keys_t[i] = _pack_rows(kT, MC).astype(BF16NP)
            v = np.zeros((tc_max * P, D), dtype=np.float32)
            v[:n] = values[g, idx, :]
            vals_c[i] = _pack_rows(v, tc_max).astype(BF16NP)
            mask_c[i, :n] = 0.0
            qt[i] = query[g]

    W1 = np.asarray(W1, dtype=np.float32)
    w1qc = _pack_jc(W1[0:M] + W1[2 * M:3 * M], MC).astype(BF16NP)
    w1bc = _pack_jc(W1[M:2 * M] - W1[2 * M:3 * M], MC).astype(BF16NP)
    w1d = _pack_jc(W1[3 * M:4 * M], MC).astype(BF16NP)
    w2p = _pack_jc(np.asarray(W2, dtype=np.float32), DC).astype(BF16NP)

    def stripe(v):
        return np.ascontiguousarray(
            np.asarray(v, dtype=np.float32).reshape(-1)[: D].reshape(DC, P).T)

    b1s = stripe(b1)
    b2s = stripe(b2)
    wss = stripe(w_score)

    shared = {
        "w1qc": w1qc, "w1bc": w1bc, "w1d": w1d, "w2": w2p,
        "b1s": b1s, "b2s": b2s, "wss": wss,
    }
    in_maps = []
    for c in range(N_CORES):
        rows = [c * B + s for s in range(B)]
        q_core = qt[rows]  # [B, M]
        qt_f = np.ascontiguousarray(
            q_core.reshape(B, MC, P).transpose(2, 1, 0).reshape(P, MC * B))
        in_maps.append({
            "qt_f": qt_f,
            "qt_b": qt_f.astype(BF16NP),
            "keys_t": np.ascontiguousarray(keys_t[rows]),
            "values": np.ascontiguousarray(vals_c[rows]),
            "mask_r": np.ascontiguousarray(mask_c[rows]),
            **shared,
        })
    return tps, slot_of, in_maps


def make_in_maps(query, keys, values, mask, W1, b1, W2, b2, w_score, b_score=None):
    # b_score is ignored: softmax is shift-invariant.
    return _prep(query, keys, values, mask, W1, b1, W2, b2, w_score)


def gather_out(results, slot_of):
    out = np.empty((64, 1, D), dtype=np.float32)
    for c in range(N_CORES):
        for s in range(B):
            out[slot_of[(c, s)], 0, :] = results[c]["out"][s]
    return out


def kernel(query, keys, values, mask, W1, b1, W2, b2, w_score, b_score):
    """Full-input entry point: shards over 8 NeuronCores, returns [64, 1, D]."""
    from concourse.bass_utils import run_bass_kernel_spmd

    tps, slot_of, in_maps = _prep(
        query, keys, values, mask, W1, b1, W2, b2, w_score)
    nc = _get_built(tps)
    res = run_bass_kernel_spmd(nc, in_maps, core_ids=list(range(N_CORES)))
    return gather_out(res.results, slot_of)
